# revision 4
# baseline (speedup 1.0000x reference)
"""Causal self-attention (B=2, T=2048, C=1024, H=16, hd=64) on 8 TRN2 NeuronCores.

Sharding: data-parallel over batch (cores 0-3 -> b=0, cores 4-7 -> b=1),
tensor-parallel over heads (4 heads per core). Each core computes its partial
out-projection; the host sums the 4 partials per batch (the "all-reduce").

Everything on-device runs in transposed space (channels on partitions) so no
transposes are ever needed:
  qkvT = w^T @ x^T          (host pre-transposes x)
  S^T[k,q] = kT^T @ qT      (heads pairs packed at partitions 0:64 / 64:128)
  P = exp(S^T / 8)          (ACT, scale fused; causal via affine_select)
  y'^T = [V|1]^T @ P^T      (ones column yields softmax denominator for free)
  outT = w_out^T @ (y^T/den)
Matmuls use float32r (TF32-like, 4x faster than fp32, ~1e-4 rel err).
"""
import sys
sys.path.insert(0, "/opt/trn_rl_repo")
import numpy as np

B, T, C = 2, 2048, 1024
N_HEAD, HD = 16, 64
HPC = 4            # heads per core
N_CORES = 8
KC = C // 128      # 8 contraction tiles for projections
NT = T // 512      # 4 q/free tiles
MT = T // 128      # 16 token tiles

_CACHE = {}


def _build_nc():
    import concourse.bass as bass  # noqa: F401
    import concourse.mybir as mybir
    import concourse.tile as tile
    from concourse import bacc

    F32, F32R = mybir.dt.float32, mybir.dt.float32r
    Exp = mybir.ActivationFunctionType.Exp
    Ident = mybir.ActivationFunctionType.Identity

    nc = bacc.Bacc("TRN2", target_bir_lowering=False, debug=False,
                   num_devices=N_CORES)
    xT_d = nc.dram_tensor("xT", [C, T], F32, kind="ExternalInput")
    wq_d = nc.dram_tensor("wq", [C, 256], F32, kind="ExternalInput")
    wk_d = nc.dram_tensor("wk", [C, 256], F32, kind="ExternalInput")
    wv_d = nc.dram_tensor("wv", [C, 256], F32, kind="ExternalInput")
    wo_d = nc.dram_tensor("wo", [256, C], F32, kind="ExternalInput")
    bq_d = nc.dram_tensor("bq", [128, 2], F32, kind="ExternalInput")
    bk_d = nc.dram_tensor("bk", [128, 2], F32, kind="ExternalInput")
    bv_d = nc.dram_tensor("bv", [128, 256], F32, kind="ExternalInput")
    bo_d = nc.dram_tensor("bo", [128, 8], F32, kind="ExternalInput")
    out_d = nc.dram_tensor("outT", [C, T], F32, kind="ExternalOutput")

    with tile.TileContext(nc) as tc:
        with (
            tc.tile_pool(name="wpool", bufs=1) as wpool,
            tc.tile_pool(name="actpool", bufs=1) as actpool,
            tc.tile_pool(name="ppool", bufs=4) as ppool,
            tc.tile_pool(name="spool", bufs=2) as spool,
            tc.tile_pool(name="opool", bufs=3) as opool,
            tc.tile_pool(name="psum", bufs=4, space="PSUM") as psum,
            tc.tile_pool(name="ypsum", bufs=2, space="PSUM") as ypsum,
        ):
            # persistent activation tiles
            # qT/kT: [128, 2, T] f32r; partition j of m-tile hp = channel of
            # head 2hp+(j//64), dim j%64
            qTr = actpool.tile([128, 2, T], F32R)
            kTr = actpool.tile([128, 2, T], F32R)
            # V natural [tok part, mt, head, 0:64] + ones col at 64
            vr = actpool.tile([128, MT, HPC, HD + 1], F32R)
            yTr = actpool.tile([128, 2, T], F32R)

            with (
                tc.tile_pool(name="xpool", bufs=2) as xpool,
                tc.tile_pool(name="stage", bufs=2) as stage,
            ):
                # ---- load weights + biases, cast to f32r ----
                def load_cast(dram, shape3, nm):
                    t32 = stage.tile(shape3, F32, tag="wstage")
                    nc.sync.dma_start(t32[:],
                                      dram.rearrange("(ko p) m -> p ko m", p=128))
                    tr = wpool.tile(shape3, F32R, tag=nm)
                    nc.vector.tensor_copy(tr[:], t32[:])
                    return tr

                wqr = load_cast(wq_d, [128, KC, 256], "wqr")
                wkr = load_cast(wk_d, [128, KC, 256], "wkr")
                wvr = load_cast(wv_d, [128, KC, 256], "wvr")
                wor = load_cast(wo_d, [128, 2, C], "wor")
                bq_sb = wpool.tile([128, 2], F32)
                nc.sync.dma_start(bq_sb[:], bq_d[:])
                bk_sb = wpool.tile([128, 2], F32)
                nc.sync.dma_start(bk_sb[:], bk_d[:])
                bv_sb = wpool.tile([128, 256], F32)
                nc.sync.dma_start(bv_sb[:], bv_d[:])
                bo_sb = wpool.tile([128, 8], F32)
                nc.sync.dma_start(bo_sb[:], bo_d[:])
                ones32 = wpool.tile([128, 1], F32)
                nc.gpsimd.memset(ones32[:], 1.0)
                nc.vector.tensor_copy(
                    vr[:, :, :, HD:HD + 1],
                    ones32[:, None, None, :].to_broadcast([128, MT, HPC, 1]))

                # ---- stream xT per nt-tile of 512 tokens; project ----
                xT_t = xT_d.rearrange("(ko p) t -> p ko t", p=128)
                for nt in range(NT):
                    xr = xpool.tile([128, KC, 512], F32R, tag="xr")
                    for ko in range(KC):
                        x32 = stage.tile([128, 512], F32, tag="xstage")
                        nc.sync.dma_start(
                            x32[:], xT_t[:, ko, nt * 512:(nt + 1) * 512])
                        nc.vector.tensor_copy(xr[:, ko], x32[:])
                    # qT / kT projections for this token slice
                    for dst, w, b in ((qTr, wqr, bq_sb), (kTr, wkr, bk_sb)):
                        for hp in range(2):
                            ps = psum.tile([128, 512], mybir.dt.float32, tag="mm")
                            for ko in range(KC):
                                nc.tensor.matmul(
                                    ps[:], w[:, ko, hp * 128:(hp + 1) * 128],
                                    xr[:, ko, :],
                                    start=(ko == 0), stop=(ko == KC - 1))
                            nc.scalar.activation(
                                dst[:, hp, nt * 512:(nt + 1) * 512], ps[:],
                                Ident, bias=b[:, hp:hp + 1])
                    # V for the 4 token tiles inside this slice
                    for mti in range(4):
                        mt = 4 * nt + mti
                        ps = psum.tile([128, 256], mybir.dt.float32, tag="mm")
                        for ko in range(KC):
                            nc.tensor.matmul(
                                ps[:], xr[:, ko, mti * 128:(mti + 1) * 128],
                                wvr[:, ko, :],
                                start=(ko == 0), stop=(ko == KC - 1))
                        nc.vector.tensor_tensor(
                            vr[:, mt, :, 0:HD],
                            ps.rearrange("p (h d) -> p h d", h=HPC),
                            bv_sb.rearrange("p (h d) -> p h d", h=HPC),
                            mybir.AluOpType.add)

            # ---- attention (per local head) ----
            for l in range(HPC):
                hp, p0 = l // 2, 64 * (l % 2)
                for qi in range(NT):
                    nkt = 4 * qi + 4
                    y_ps = ypsum.tile([HD + 1, 512], mybir.dt.float32)
                    for kt in range(nkt):
                        s_ps = psum.tile([128, 512], mybir.dt.float32, tag="mm")
                        nc.tensor.matmul(
                            s_ps[:],
                            kTr[p0:p0 + 64, hp, kt * 128:(kt + 1) * 128],
                            qTr[p0:p0 + 64, hp, qi * 512:(qi + 1) * 512],
                            start=True, stop=True)
                        p_sb = ppool.tile([128, 512], F32R)
                        nc.scalar.activation(p_sb[:], s_ps[:], Exp, scale=0.125)
                        if kt >= 4 * qi:  # diagonal block: causal mask
                            nc.gpsimd.affine_select(
                                p_sb[:], p_sb[:], pattern=[[1, 512]],
                                compare_op=mybir.AluOpType.is_ge, fill=0.0,
                                base=512 * qi - 128 * kt, channel_multiplier=-1)
                        nc.tensor.matmul(y_ps[:], vr[:, kt, l, :], p_sb[:],
                                         start=(kt == 0), stop=(kt == nkt - 1))
                    recip = spool.tile([1, 512], F32, tag="recip")
                    nc.vector.reciprocal(recip[:], y_ps[HD:HD + 1, :])
                    rb = spool.tile([64, 512], F32, tag="rb")
                    nc.gpsimd.partition_broadcast(rb[:], recip[:])
                    nc.vector.tensor_tensor(
                        yTr[p0:p0 + 64, hp, qi * 512:(qi + 1) * 512],
                        y_ps[0:HD, :], rb[:], mybir.AluOpType.mult)

            # ---- out projection (partial; host sums across 4 cores) ----
            for m in range(8):
                for nt in range(NT):
                    ps = psum.tile([128, 512], mybir.dt.float32, tag="mm")
                    for kt in range(2):
                        nc.tensor.matmul(
                            ps[:], wor[:, kt, m * 128:(m + 1) * 128],
                            yTr[:, kt, nt * 512:(nt + 1) * 512],
                            start=(kt == 0), stop=(kt == 1))
                    o_sb = opool.tile([128, 512], F32)
                    nc.scalar.activation(o_sb[:], ps[:], Ident,
                                         bias=bo_sb[:, m:m + 1])
                    nc.sync.dma_start(
                        out_d[m * 128:(m + 1) * 128, nt * 512:(nt + 1) * 512],
                        o_sb[:])
    nc.compile()
    return nc


def _prep_in_maps(x, w_qkv, b_qkv, w_out, b_out):
    """Slice/transpose full inputs into per-core arrays."""
    x = np.asarray(x, dtype=np.float32)
    w_qkv = np.asarray(w_qkv, dtype=np.float32)
    b_qkv = np.asarray(b_qkv, dtype=np.float32)
    w_out = np.asarray(w_out, dtype=np.float32)
    b_out = np.asarray(b_out, dtype=np.float32)

    xT = [np.ascontiguousarray(x[b].T) for b in range(B)]
    # w_qkv columns: head h occupies cols [h*3*HD, (h+1)*3*HD): q=0:64 k=64:128 v=128:192
    wq3 = w_qkv.reshape(C, N_HEAD, 3 * HD)
    bq3 = b_qkv.reshape(N_HEAD, 3 * HD)
    in_maps = []
    for c in range(N_CORES):
        b = c // 4
        g0 = HPC * (c % 4)  # first global head on this core
        gl = [g0 + l for l in range(HPC)]
        # wq/wk: [C, 2, 128]: m-tile hp packs heads 2hp, 2hp+1 (64 dims each)
        wq = np.concatenate([wq3[:, g, 0:HD] for g in gl], axis=1)
        wk = np.concatenate([wq3[:, g, HD:2 * HD] for g in gl], axis=1)
        wv = np.concatenate([wq3[:, g, 2 * HD:3 * HD] for g in gl], axis=1)
        wo = np.concatenate([w_out[g * HD:(g + 1) * HD, :] for g in gl], axis=0)
        bq = np.concatenate([bq3[g, 0:HD] for g in gl]).reshape(2, 128).T
        bk = np.concatenate([bq3[g, HD:2 * HD] for g in gl]).reshape(2, 128).T
        bv = np.broadcast_to(
            np.concatenate([bq3[g, 2 * HD:3 * HD] for g in gl]), (128, 256))
        bo = (b_out if c % 4 == 0 else np.zeros_like(b_out)).reshape(8, 128).T
        in_maps.append({
            "xT": xT[b],
            "wq": np.ascontiguousarray(wq), "wk": np.ascontiguousarray(wk),
            "wv": np.ascontiguousarray(wv), "wo": np.ascontiguousarray(wo),
            "bq": np.ascontiguousarray(bq), "bk": np.ascontiguousarray(bk),
            "bv": np.ascontiguousarray(bv), "bo": np.ascontiguousarray(bo),
        })
    return in_maps


def _get_runner():
    """Build (once) a jitted 8-core SPMD callable. Returns (fn, meta) where
    fn(concat_inputs: list[np.ndarray|jax.Array]) -> list of per-core outT."""
    if "runner" in _CACHE:
        return _CACHE["runner"]
    import jax
    import numpy as np
    from jax.sharding import Mesh, PartitionSpec
    from jax.experimental.shard_map import shard_map
    import concourse.mybir as mybir
    from concourse.bass2jax import (_bass_exec_p, install_neuronx_cc_hook,
                                    partition_id_tensor)

    nc = _build_nc()
    install_neuronx_cc_hook()

    in_names, out_names, out_avals = [], [], []
    for alloc in nc.m.functions[0].allocations:
        if not isinstance(alloc, mybir.MemoryLocationSet):
            continue
        name = alloc.memorylocations[0].name
        if alloc.kind == "ExternalInput":
            if nc.partition_id_tensor is None or name != nc.partition_id_tensor.name:
                in_names.append(name)
        elif alloc.kind == "ExternalOutput":
            out_names.append(name)
            out_avals.append(jax.core.ShapedArray(
                tuple(alloc.tensor_shape), mybir.dt.np(alloc.dtype)))
    n_params = len(in_names)
    all_names = in_names + out_names
    partition_name = (nc.partition_id_tensor.name
                      if nc.partition_id_tensor else None)
    if partition_name is not None:
        all_names = all_names + [partition_name]

    def _body(*args):
        operands = list(args)
        if partition_name is not None:
            operands.append(partition_id_tensor())
        return tuple(_bass_exec_p.bind(
            *operands,
            out_avals=tuple(out_avals),
            in_names=tuple(all_names),
            out_names=tuple(out_names),
            lowering_input_output_aliases=(),
            sim_require_finite=True,
            sim_require_nnan=True,
            nc=nc))

    devices = jax.devices()[:N_CORES]
    mesh = Mesh(np.asarray(devices), ("core",))
    n_out = len(out_names)
    fn = jax.jit(shard_map(
        _body, mesh=mesh,
        in_specs=(PartitionSpec("core"),) * (n_params + n_out),
        out_specs=(PartitionSpec("core"),) * n_out))

    _CACHE["runner"] = (fn, in_names, out_names, out_avals, mesh)
    return _CACHE["runner"]


def _concat_inputs(in_maps, in_names, out_avals):
    concat_in = [np.concatenate([m[nm] for m in in_maps], axis=0)
                 for nm in in_names]
    zeros = [np.zeros((N_CORES * av.shape[0],) + tuple(av.shape[1:]), av.dtype)
             for av in out_avals]
    return concat_in + zeros


def _assemble(out_concat):
    """out_concat: [8*C, T] array of per-core outT partials -> [B, T, C]."""
    o = np.asarray(out_concat).reshape(N_CORES, C, T)
    out = np.zeros((B, T, C), dtype=np.float32)
    for c in range(N_CORES):
        out[c // 4] += o[c].T
    return out


def kernel(**inputs):
    fn, in_names, out_names, out_avals, mesh = _get_runner()
    in_maps = _prep_in_maps(**inputs)
    args = _concat_inputs(in_maps, in_names, out_avals)
    outs = fn(*args)
    return _assemble(outs[0])


if __name__ == "__main__":
    # smoke test with random data
    rng = np.random.default_rng(0)
    s = 1.0 / np.sqrt(C)
    ins = {
        "x": rng.standard_normal((B, T, C)).astype(np.float32),
        "w_qkv": (rng.standard_normal((C, 3 * C)) * s).astype(np.float32),
        "b_qkv": np.zeros(3 * C, np.float32),
        "w_out": (rng.standard_normal((C, C)) * s).astype(np.float32),
        "b_out": np.zeros(C, np.float32),
    }
    out = kernel(**ins)
    print("out", out.shape, out.dtype, float(np.abs(out).max()))


# revision 27
# speedup vs baseline: 523.8544x; 523.8544x over previous
"""Causal self-attention (B=2, T=2048, C=1024, H=16, hd=64) on 8 TRN2 NeuronCores.

Sharding: data-parallel over batch (cores 0-3 -> b=0, cores 4-7 -> b=1),
tensor-parallel over heads (4 heads per core). Each core computes its partial
out-projection; the host sums the 4 partials per batch (the "all-reduce").

Everything on-device runs in transposed space (channels on partitions) so no
transposes are ever needed:
  qkvT = w^T @ x^T          (host pre-transposes x)
  S^T[k,q] = kT^T @ qT      (head pairs packed at partitions 0:64 / 64:128)
  P = exp(S^T / 8)          (ACT, scale fused; causal via affine_select on
                             diagonal blocks only; sub-diagonal blocks skipped,
                             diagonal blocks column-narrowed)
  y'^T = [V|1]^T @ P^T      (ones column yields softmax denominator for free)
  outT = w_out^T @ (y^T/den)
Matmuls use float32r (TF32-like, 4x faster than fp32, ~1e-4 rel err); DRAM
inputs are declared float32r so DMA loads need no rounding pass.
"""
import sys
sys.path.insert(0, "/opt/trn_rl_repo")
import numpy as np

B, T, C = 2, 2048, 1024
N_HEAD, HD = 16, 64
HPC = 4            # heads per core
N_CORES = 8
KC = C // 128      # 8 contraction tiles for projections
NT = T // 512      # 4 q/free tiles
MT = T // 128      # 16 token tiles

_CACHE = {}


def _build_nc(use_bias, reps=1, reload_x=True):
    import concourse.bass as bass  # noqa: F401
    import concourse.mybir as mybir
    import concourse.tile as tile
    from concourse import bacc

    F32, F32R = mybir.dt.float32, mybir.dt.float32r
    Exp = mybir.ActivationFunctionType.Exp
    Ident = mybir.ActivationFunctionType.Identity

    nc = bacc.Bacc("TRN2", target_bir_lowering=False, debug=False,
                   num_devices=N_CORES)
    xT_d = nc.dram_tensor("xT", [C, T], F32R, kind="ExternalInput")
    wq_d = nc.dram_tensor("wq", [C, 256], F32R, kind="ExternalInput")
    wk_d = nc.dram_tensor("wk", [C, 256], F32R, kind="ExternalInput")
    wv_d = nc.dram_tensor("wv", [C, 256], F32R, kind="ExternalInput")
    wo_d = nc.dram_tensor("wo", [256, C], F32R, kind="ExternalInput")
    if use_bias:
        bq_d = nc.dram_tensor("bq", [128, 2], F32, kind="ExternalInput")
        bk_d = nc.dram_tensor("bk", [128, 2], F32, kind="ExternalInput")
        bv_d = nc.dram_tensor("bv", [128, 256], F32, kind="ExternalInput")
        bo_d = nc.dram_tensor("bo", [128, 8], F32, kind="ExternalInput")
    out_d = nc.dram_tensor("outT", [C, T], F32, kind="ExternalOutput")

    with tile.TileContext(nc) as tc:
        with (
            tc.tile_pool(name="wpool", bufs=1) as wpool,
            tc.tile_pool(name="actpool", bufs=1) as actpool,
            tc.tile_pool(name="ppool", bufs=6) as ppool,
            tc.tile_pool(name="spool", bufs=2) as spool,
            tc.tile_pool(name="opool", bufs=4) as opool,
            tc.tile_pool(name="xpool", bufs=(2 if reload_x else 4)) as xpool,
            tc.tile_pool(name="psum", bufs=2, space="PSUM") as psum,
            tc.tile_pool(name="mmpsum", bufs=2, space="PSUM") as mmpsum,
            tc.tile_pool(name="ypsum", bufs=2, space="PSUM") as ypsum,
        ):
            # persistent activation tiles
            # qT/kT: [128, 2, T] f32r; partition j of m-tile hp = channel of
            # head 2hp+(j//64), dim j%64
            qTr = actpool.tile([128, 2, T], F32R)
            kTr = actpool.tile([128, 2, T], F32R)
            # V natural [tok part, mt, head, 0:64] + ones col at 64
            vr = actpool.tile([128, MT, HPC, HD + 1], F32R)
            yTr = actpool.tile([128, 2, T], F32R)

            # ---- weights + biases (direct f32r DMA, no casts) ----
            # DMA priority order: wq, x(0) (emitted by caller right after),
            # then wk/wv, wo last (only needed at out-proj time).
            xT_t0 = xT_d.rearrange("(ko p) t -> p ko t", p=128)
            wqr = wpool.tile([128, KC, 256], F32R, tag="wqr")
            wq_t = wq_d.rearrange("(ko p) m -> p ko m", p=128)
            nc.sync.dma_start(wqr[:, 0:4], wq_t[:, 0:4])
            xr0 = xpool.tile([128, KC, 512], F32R, tag="xr", name="xr")
            nc.sync.dma_start(xr0[:, 0:4], xT_t0[:, 0:4, 0:512])
            nc.sync.dma_start(wqr[:, 4:8], wq_t[:, 4:8])
            nc.sync.dma_start(xr0[:, 4:8], xT_t0[:, 4:8, 0:512])
            wkr = wpool.tile([128, KC, 256], F32R, tag="wkr")
            nc.sync.dma_start(wkr[:], wk_d.rearrange("(ko p) m -> p ko m", p=128))
            wvr = wpool.tile([128, KC, 256], F32R, tag="wvr")
            nc.sync.dma_start(wvr[:], wv_d.rearrange("(ko p) m -> p ko m", p=128))
            wor = wpool.tile([128, 2, C], F32R, tag="wor")
            nc.sync.dma_start(wor[:], wo_d.rearrange("(ko p) m -> p ko m", p=128))
            if use_bias:
                bq_sb = wpool.tile([128, 2], F32, tag="bq")
                nc.sync.dma_start(bq_sb[:], bq_d[:])
                bk_sb = wpool.tile([128, 2], F32, tag="bk")
                nc.sync.dma_start(bk_sb[:], bk_d[:])
                bv_sb = wpool.tile([128, 256], F32, tag="bv")
                nc.sync.dma_start(bv_sb[:], bv_d[:])
                bo_sb = wpool.tile([128, 8], F32, tag="bo")
                nc.sync.dma_start(bo_sb[:], bo_d[:])
            ones32 = wpool.tile([128, 1], F32, tag="ones")
            nc.gpsimd.memset(ones32[:], 1.0)
            nc.vector.tensor_copy(
                vr[:, :, :, HD:HD + 1],
                ones32[:, None, None, :].to_broadcast([128, MT, HPC, 1]))

            # PSUM: "pp" pair tiles [128,1024] (2 banks) x2 + "yp" [65,1024]
            # (2 banks) x2 = 8 banks exactly.
            def pp_tile():
                return psum.tile([128, 2, 512], mybir.dt.float32, tag="pp", name="pp")

            def mm_tile():
                return mmpsum.tile([128, 512], mybir.dt.float32, tag="mm", name="mm")

            xT_t = xT_d.rearrange("(ko p) t -> p ko t", p=128)

            def load_x(nt):
                xr = xpool.tile([128, KC, 512], F32R, tag="xr", name="xr")
                nc.sync.dma_start(xr[:], xT_t[:, :, nt * 512:(nt + 1) * 512])
                return xr

            def proj_chunks(nt, xr):
                """q/k/v projections for token slice nt; yields per psum chain."""
                for dst, w, bias_nm in ((qTr, wqr, "bq"), (kTr, wkr, "bk")):
                    for hp in range(2):
                        ps = mm_tile()[:]
                        for ko in range(KC):
                            nc.tensor.matmul(
                                ps, w[:, ko, hp * 128:(hp + 1) * 128],
                                xr[:, ko, :],
                                start=(ko == 0), stop=(ko == KC - 1))
                        dslice = dst[:, hp, nt * 512:(nt + 1) * 512]
                        if use_bias:
                            b_sb = bq_sb if bias_nm == "bq" else bk_sb
                            nc.scalar.activation(dslice, ps, Ident,
                                                 bias=b_sb[:, hp:hp + 1])
                        else:
                            nc.vector.tensor_copy(dslice, ps)
                        yield
                for mti in range(4):
                    mt = 4 * nt + mti
                    ps = mm_tile()[:, 0:256]
                    for ko in range(KC):
                        nc.tensor.matmul(
                            ps, xr[:, ko, mti * 128:(mti + 1) * 128],
                            wvr[:, ko, :],
                            start=(ko == 0), stop=(ko == KC - 1))
                    if use_bias:
                        nc.vector.tensor_tensor(
                            vr[:, mt, :, 0:HD],
                            ps.rearrange("p (h d) -> p h d", h=HPC),
                            bv_sb.rearrange("p (h d) -> p h d", h=HPC),
                            mybir.AluOpType.add)
                    else:
                        nc.vector.tensor_copy(
                            vr[:, mt, :, 0:HD],
                            ps.rearrange("p (h d) -> p h d", h=HPC))
                    yield

            def attention_blocks(hp, qi):
                """head pair hp (h0 at partitions 0:64, h1 at 64:128), q-tile
                qi; yields per S/exp/PV block; retire emitted at the end."""
                h0, h1 = 2 * hp, 2 * hp + 1
                nkt = 4 * qi + 4
                y_ps = [ypsum.tile([HD + 1, 512], mybir.dt.float32,
                                   tag="yp", name="yp") for _ in range(2)]
                for kt in range(nkt):
                    diag = kt >= 4 * qi
                    c0_true = 128 * kt - 512 * qi if diag else 0
                    # f32r matmuls need moving dim >=256 for 1 cyc/row; never
                    # narrow below that
                    c0 = min(c0_true, 256)
                    w_ = 512 - c0
                    s_ps = pp_tile()
                    # S^T blocks for both heads of the pair -> adjacent
                    # matmuls on disjoint PE row groups (0:64 / 64:128)
                    for i, p0 in ((0, 0), (1, 64)):
                        nc.tensor.matmul(
                            s_ps[:, i, c0:512],
                            kTr[p0:p0 + 64, hp, kt * 128:(kt + 1) * 128],
                            qTr[p0:p0 + 64, hp,
                                qi * 512 + c0:(qi + 1) * 512],
                            start=True, stop=True)
                    p_sb = ppool.tile([128, 2, 512], F32R, tag="p", name="p")
                    nc.scalar.activation(p_sb[:, :, c0:512],
                                         s_ps[:, :, c0:512], Exp,
                                         scale=0.125)
                    if diag:
                        # keep q >= k: f - p + (c0 - c0_true) >= 0
                        nc.gpsimd.affine_select(
                            p_sb[:, :, c0:512], p_sb[:, :, c0:512],
                            pattern=[[0, 2], [1, w_]],
                            compare_op=mybir.AluOpType.is_ge, fill=0.0,
                            base=c0 - c0_true, channel_multiplier=-1)
                    for i, l in ((0, h0), (1, h1)):
                        nc.tensor.matmul(
                            y_ps[i][:, c0:512], vr[:, kt, l, :],
                            p_sb[:, i, c0:512],
                            start=(kt == 0), stop=(kt == nkt - 1))
                    yield
                for i, p0 in ((0, 0), (1, 64)):
                    recip = spool.tile([1, 512], F32, tag="recip", name="recip")
                    nc.vector.reciprocal(recip[:], y_ps[i][HD:HD + 1, :])
                    rb = spool.tile([64, 512], F32, tag="rb", name="rb")
                    nc.gpsimd.partition_broadcast(rb[:], recip[:])
                    nc.vector.tensor_tensor(
                        yTr[p0:p0 + 64, hp, qi * 512:(qi + 1) * 512],
                        y_ps[i][0:HD, :], rb[:], mybir.AluOpType.mult)

            def out_chunks(nt):
                """partial out-projection for token columns nt; yields per m."""
                for m in range(8):
                    ps = mm_tile()[:]
                    for kt in range(2):
                        nc.tensor.matmul(
                            ps, wor[:, kt, m * 128:(m + 1) * 128],
                            yTr[:, kt, nt * 512:(nt + 1) * 512],
                            start=(kt == 0), stop=(kt == 1))
                    o_sb = opool.tile([128, 512], F32, tag="o", name="o")
                    if use_bias:
                        nc.scalar.activation(o_sb[:], ps, Ident,
                                             bias=bo_sb[:, m:m + 1])
                    else:
                        nc.vector.tensor_copy(o_sb[:], ps)
                    nc.sync.dma_start(
                        out_d[m * 128:(m + 1) * 128,
                              nt * 512:(nt + 1) * 512], o_sb[:])
                    yield

            def drain(gen, n=None):
                k = 0
                for _ in gen:
                    k += 1
                    if n is not None and k >= n:
                        return True
                return False

            # ---- fused + interleaved pipeline ----
            # Per step nt: attention blocks of both head pairs alternate
            # (two independent S->exp->PV chains keep PE fed through exp
            # latency), with proj(nt+1) and out_proj(nt-1) chunks spread
            # between them so PE always has slot-granted work during the
            # ACT-bound attention stretches.
            xr_cache = {0: xr0}
            for rep in range(reps):
                if rep == 0:
                    xr = xr0
                elif reload_x:
                    xr = load_x(0)
                else:
                    xr = xr_cache[0]
                drain(proj_chunks(0, xr))
                for nt in range(NT):
                    if nt + 1 >= NT:
                        xr = None
                    elif rep == 0:
                        xr = load_x(nt + 1)
                        xr_cache[nt + 1] = xr
                    elif reload_x:
                        xr = load_x(nt + 1)
                    else:
                        xr = xr_cache[nt + 1]
                    aux = []
                    if xr is not None:
                        aux.append(proj_chunks(nt + 1, xr))
                    if nt > 0:
                        aux.append(out_chunks(nt - 1))
                    gens = [attention_blocks(0, nt)]
                    gens2 = [attention_blocks(1, nt)]
                    nblocks = 2 * (4 * nt + 4)
                    naux = 8 + (8 if nt > 0 else 0)
                    emitted = 0
                    live = list(gens)
                    ai = 0
                    while live or gens2:
                        if not live:
                            live, gens2 = gens2, []
                        for g in list(live):
                            if not drain(g, 1):
                                live.remove(g)
                                continue
                            emitted += 1
                            # spread aux chunks evenly across blocks
                            want = (emitted * naux) // max(nblocks, 1)
                            while ai < want and aux:
                                if not drain(aux[ai % len(aux)], 1):
                                    aux.pop(ai % len(aux))
                                else:
                                    ai += 1
                    for g in aux:
                        drain(g)
                out_chunks_left = out_chunks(NT - 1)
                drain(out_chunks_left)
    nc.compile()
    return nc


def _prep_in_maps(x, w_qkv, b_qkv, w_out, b_out, use_bias):
    """Slice/transpose full inputs into per-core arrays."""
    x = np.asarray(x, dtype=np.float32)
    w_qkv = np.asarray(w_qkv, dtype=np.float32)
    b_qkv = np.asarray(b_qkv, dtype=np.float32)
    w_out = np.asarray(w_out, dtype=np.float32)
    b_out = np.asarray(b_out, dtype=np.float32)

    xT = [np.ascontiguousarray(x[b].T) for b in range(B)]
    # w_qkv columns: head h occupies cols [h*3*HD, (h+1)*3*HD): q=0:64 k=64:128 v=128:192
    wq3 = w_qkv.reshape(C, N_HEAD, 3 * HD)
    bq3 = b_qkv.reshape(N_HEAD, 3 * HD)
    in_maps = []
    for c in range(N_CORES):
        b = c // 4
        g0 = HPC * (c % 4)  # first global head on this core
        gl = [g0 + l for l in range(HPC)]
        # wq/wk: [C, 2, 128]: m-tile hp packs heads 2hp, 2hp+1 (64 dims each)
        wq = np.concatenate([wq3[:, g, 0:HD] for g in gl], axis=1)
        wk = np.concatenate([wq3[:, g, HD:2 * HD] for g in gl], axis=1)
        wv = np.concatenate([wq3[:, g, 2 * HD:3 * HD] for g in gl], axis=1)
        wo = np.concatenate([w_out[g * HD:(g + 1) * HD, :] for g in gl], axis=0)
        m = {
            "xT": xT[b],
            "wq": np.ascontiguousarray(wq), "wk": np.ascontiguousarray(wk),
            "wv": np.ascontiguousarray(wv), "wo": np.ascontiguousarray(wo),
        }
        if use_bias:
            m["bq"] = np.ascontiguousarray(
                np.concatenate([bq3[g, 0:HD] for g in gl]).reshape(2, 128).T)
            m["bk"] = np.ascontiguousarray(
                np.concatenate([bq3[g, HD:2 * HD] for g in gl]).reshape(2, 128).T)
            m["bv"] = np.ascontiguousarray(np.broadcast_to(
                np.concatenate([bq3[g, 2 * HD:3 * HD] for g in gl]), (128, 256)))
            m["bo"] = np.ascontiguousarray(
                (b_out if c % 4 == 0 else np.zeros_like(b_out)).reshape(8, 128).T)
        in_maps.append(m)
    return in_maps


def _get_runner(use_bias=False, reps=1, reload_x=True):
    """Build (once) a jitted 8-core SPMD callable."""
    key = ("runner", use_bias, reps, reload_x)
    if key in _CACHE:
        return _CACHE[key]
    import jax
    from jax.sharding import Mesh, PartitionSpec
    from jax.experimental.shard_map import shard_map
    import concourse.mybir as mybir
    from concourse.bass2jax import (_bass_exec_p, install_neuronx_cc_hook,
                                    partition_id_tensor)

    nc = _build_nc(use_bias, reps, reload_x)
    install_neuronx_cc_hook()

    in_names, out_names, out_avals = [], [], []
    for alloc in nc.m.functions[0].allocations:
        if not isinstance(alloc, mybir.MemoryLocationSet):
            continue
        name = alloc.memorylocations[0].name
        if alloc.kind == "ExternalInput":
            if nc.partition_id_tensor is None or name != nc.partition_id_tensor.name:
                in_names.append(name)
        elif alloc.kind == "ExternalOutput":
            out_names.append(name)
            out_avals.append(jax.core.ShapedArray(
                tuple(alloc.tensor_shape), mybir.dt.np(alloc.dtype)))
    n_params = len(in_names)
    all_names = in_names + out_names
    partition_name = (nc.partition_id_tensor.name
                      if nc.partition_id_tensor else None)
    if partition_name is not None:
        all_names = all_names + [partition_name]

    def _body(*args):
        operands = list(args)
        if partition_name is not None:
            operands.append(partition_id_tensor())
        return tuple(_bass_exec_p.bind(
            *operands,
            out_avals=tuple(out_avals),
            in_names=tuple(all_names),
            out_names=tuple(out_names),
            lowering_input_output_aliases=(),
            sim_require_finite=True,
            sim_require_nnan=True,
            nc=nc))

    devices = jax.devices()[:N_CORES]
    mesh = Mesh(np.asarray(devices), ("core",))
    n_out = len(out_names)
    fn = jax.jit(shard_map(
        _body, mesh=mesh,
        in_specs=(PartitionSpec("core"),) * (n_params + n_out),
        out_specs=(PartitionSpec("core"),) * n_out))

    _CACHE[key] = (fn, in_names, out_names, out_avals, mesh)
    return _CACHE[key]


def _concat_inputs(in_maps, in_names, out_avals):
    concat_in = [np.concatenate([m[nm] for m in in_maps], axis=0)
                 for nm in in_names]
    zeros = [np.zeros((N_CORES * av.shape[0],) + tuple(av.shape[1:]), av.dtype)
             for av in out_avals]
    return concat_in + zeros


def _assemble(out_concat):
    """out_concat: [8*C, T] array of per-core outT partials -> [B, T, C]."""
    o = np.asarray(out_concat).reshape(B, 4, C, T)
    return o.sum(axis=1, dtype=np.float32).transpose(0, 2, 1)


def kernel(**inputs):
    use_bias = bool(np.any(np.asarray(inputs["b_qkv"]))
                    or np.any(np.asarray(inputs["b_out"])))
    fn, in_names, out_names, out_avals, mesh = _get_runner(use_bias)
    in_maps = _prep_in_maps(use_bias=use_bias, **inputs)
    args = _concat_inputs(in_maps, in_names, out_avals)
    outs = fn(*args)
    return _assemble(outs[0])


if __name__ == "__main__":
    rng = np.random.default_rng(0)
    s = 1.0 / np.sqrt(C)
    ins = {
        "x": rng.standard_normal((B, T, C)).astype(np.float32),
        "w_qkv": (rng.standard_normal((C, 3 * C)) * s).astype(np.float32),
        "b_qkv": np.zeros(3 * C, np.float32),
        "w_out": (rng.standard_normal((C, C)) * s).astype(np.float32),
        "b_out": np.zeros(C, np.float32),
    }
    out = kernel(**ins)
    print("out", out.shape, out.dtype, float(np.abs(out).max()))
